# revision 17
# baseline (speedup 1.0000x reference)
"""CostVolumeLayer Trainium2 kernel (v3).

Computes the local cost volume: for search_range R=4,
  out[b, di*9+dj, i, j] = sum_c src[b,c,i,j] * tgt_zp[b,c,i-2R+di, j-2R+dj]
(tgt zero-padded outside its bounds; off-center window, faithful to the
torch reference).

Strategy (8 NeuronCores, SPMD):
  - Shard: core c -> batch b = c//2, row-half r0 = 32*(c%2).
  - Device: per 8x16 pixel block, TWO accumulating TensorE matmuls build
    the banded Gram at full 128-partition occupancy in ONE psum bank:
      A: stationary [pixels mi<4 | zeros],  rhs = window rows
         [8bi, 8bi+12) x 24 cols (N=288), start=True
      B: stationary [zeros | pixels mi>=4], rhs = rows [8bi+4, 8bi+16),
         start=False (accumulates; each writes zeros to the other half).
    The zero half comes from a single shared 64-col zero strip at the
    start of the input tile, addressed via a two-run access pattern
    (stride -pA / +pB), so src costs no duplicate bytes.
  - The DMA fabric is the bottleneck (~430 GB/s shared across ALL
    queues, measured): total traffic is 2.47 MB in + 2.36 MB out per
    core. Inputs split across the two HWDGE queues (sync: src, scalar:
    tgt); most output DMAs go through the otherwise-idle GpSimd SWDGE
    so the HWDGE queues and ACT stay free for inputs + psum copies.
  - PSUM: 4-bank tiles (4 blocks), 2 bufs = all 8 banks; one fp32->fp16
    copy per tile [128, 4x288] (DVE x5 / ACT x3); per-tile SBUF stage
    [128, 1152] -> one output DMA each (8 total).
  - PE warm-up fills the gap until the first input chunks land; first
    chunks are small so real matmuls start ~9.5us.
  - Host: zero-FLOP banded-diagonal gather (band layout identical to
    the baseline).
"""

import numpy as np

R = 4
D = 2 * R + 1          # 9
B, C, H, W = 4, 128, 64, 128
NCORES = 8
HS = H // 2            # 32 rows per core shard
TH = HS + 2 * R        # 40 padded tgt rows per shard
TW = W + 2 * R         # 136 padded tgt cols
BI, BJ = 8, 16         # pixel block: 8 rows x 16 cols = 128 = M
NBI, NBJ = HS // BI, W // BJ   # 4 x 8 = 32 blocks per core
WIN_J = BJ + 2 * R     # 24 window cols
NA = 12 * WIN_J        # 288 streamed cols per half-matmul
BANDW = NA             # 288 band cols dumped per pixel
BANDO = 4 * WIN_J      # 96, upper-half band column offset (host gather)
BLKC = 192             # src cols per block: [pixA 64 | zeros 64 | pixB 64]
SRCC = NBI * NBJ * BLKC  # 6144 src cols
TGT0 = SRCC
TGTC = TH * TW           # 5440
E = TGT0 + TGTC          # 11584 input cols per partition
PSB = 512              # fp32 elems per PSUM bank (2KB)
TPB = 4                # blocks (banks) per psum tile
STGW = TPB * BANDW     # 1152 fp16 cols per stage
NT = NBI * NBJ // TPB  # 8 psum tiles / output stages

_compiled = None


def _build_bass():
    import concourse.mybir as mybir
    from concourse import bacc
    from concourse.tile import TileContext

    f32 = mybir.dt.float32
    in_dt = mybir.dt.bfloat16
    dump_dt = mybir.dt.float16
    nc = bacc.Bacc()
    inp = nc.dram_tensor("inp", [C, E], in_dt, kind="ExternalInput")
    gout = nc.dram_tensor("gout", [NT, 128, STGW], dump_dt,
                          kind="ExternalOutput")
    gout_ap = gout.ap()

    with TileContext(nc) as tc:
        with (
            tc.tile_pool(name="inp", bufs=1) as inp_pool,
            tc.tile_pool(name="g", bufs=NT) as gpool,
            tc.tile_pool(name="psum", bufs=2, space="PSUM") as psum_pool,
        ):
            a = inp_pool.tile([C, E], in_dt)

            def t_view():
                return a[:, TGT0:].rearrange("c (i j) -> c i j", j=TW)

            def lhs_ap(blk, half):
                """Contiguous stationary windows over the per-block layout
                [pixA 64 | zeros 64 | pixB 64]: A=[pixA|z], B=[z|pixB].
                (Walrus rejects multi-run weight APs, so the zero strip is
                interleaved per block on the host instead.)"""
                sb = blk * BLKC + half * 64
                return a[:, sb:sb + 128]

            def new_pt():
                return psum_pool.tile([128, TPB * PSB], f32, name="pt")

            warm = inp_pool.tile([128, PSB], in_dt)
            nc.gpsimd.memset(warm, 0.0)
            # PE warm-up: HAM clock-gate ramp needs sustained PE activity;
            # fill the wait for the first input chunks. Warm matmuls write
            # into the first real psum tile; block 0's start=True matmul
            # overwrites (PE program order keeps this safe).
            wps = new_pt()
            for _ in range(8):
                nc.tensor.matmul(wps[0:1, 0:PSB], warm[:, :1], warm,
                                 start=True, stop=True)
            # ACT warm-up: first Activation op loads the activation table.
            actwarm = inp_pool.tile([1, 1], dump_dt)
            nc.scalar.copy(actwarm, warm[0:1, 0:1])

            # Input DMAs: ALL on the sync queue, in consumption order.
            # The DMA fabric (~430 GB/s) is shared by every queue, and one
            # queue alone saturates it — a single FIFO makes the first-
            # needed bytes complete first instead of interleaving with
            # later chunks.
            iv = inp.ap()

            def chunk(lo, hi):
                nc.sync.dma_start(out=a[:, lo:hi], in_=iv[:, lo:hi])

            def tchunk(r0, r1):
                chunk(TGT0 + r0 * TW, TGT0 + r1 * TW)

            # Per-DMA-instruction cost on a queue is ~0.65us regardless of
            # size, so use few, large chunks in consumption order.
            chunk(0, 8 * BLKC)            # blocks 0-7   (tiles 0-1)
            tchunk(0, 16)                 # tgt rows 0-15  (bi=0)
            chunk(8 * BLKC, 16 * BLKC)    # blocks 8-15  (tiles 2-3)
            tchunk(16, 28)                # rows 16-27     (bi=1, bi=2 A)
            chunk(16 * BLKC, 24 * BLKC)   # blocks 16-23 (tiles 4-5)
            tchunk(28, 40)                # rows 28-39     (bi=2 B, bi=3)
            chunk(24 * BLKC, TGT0)        # blocks 24-31 (tiles 6-7)

            for t in range(NT):
                bi, h = divmod(t, 2)
                stage = gpool.tile([128, STGW], dump_dt)
                pt = wps if t == 0 else new_pt()
                ptv = pt.rearrange("p (b h) -> p b h", b=TPB)
                for j in range(TPB):
                    blk = t * TPB + j
                    bj = blk % NBJ
                    rhsA = t_view()[:, bi * BI: bi * BI + 12,
                                    bj * BJ: bj * BJ + WIN_J]
                    rhsB = t_view()[:, bi * BI + 4: bi * BI + 16,
                                    bj * BJ: bj * BJ + WIN_J]
                    nc.tensor.matmul(ptv[:, j, :NA], lhs_ap(blk, 0), rhsA,
                                     start=True, stop=False)
                    nc.tensor.matmul(ptv[:, j, :NA], lhs_ap(blk, 1), rhsB,
                                     start=False, stop=True)
                dstv = stage.rearrange("p (b w) -> p b w", b=TPB)
                # every tile's evacuation split across BOTH engines: psum
                # freed in ~0.75us instead of ~1.36 (kills tile stalls and
                # shortens the tail)
                nc.vector.tensor_copy(dstv[:, 0:2], ptv[:, 0:2, 0:BANDW])
                nc.scalar.copy(dstv[:, 2:4], ptv[:, 2:4, 0:BANDW])
                if t < NT - 1:
                    # bulk outputs via the otherwise-idle GpSimd SWDGE;
                    # both HWDGE queues stay clear (sync: inputs, scalar:
                    # ACT copies + the tail DMA)
                    nc.gpsimd.dma_start(out=gout_ap[t], in_=stage)
                else:
                    # tail-latency: ship the last tile's halves on the two
                    # HWDGE queues in parallel
                    half = STGW // 2
                    nc.sync.dma_start(out=gout_ap[t][:, 0:half],
                                      in_=stage[:, 0:half])
                    nc.scalar.dma_start(out=gout_ap[t][:, half:],
                                        in_=stage[:, half:])
    nc.finalize()
    return nc


def _get_compiled():
    global _compiled
    if _compiled is None:
        _compiled = _build_bass()
    return _compiled


def _shard_inputs(src, tgt):
    """Per-core input maps: [zero strip | block-reordered src | padded tgt]."""
    import ml_dtypes

    bf16 = ml_dtypes.bfloat16
    in_maps = []
    for c in range(NCORES):
        b = c // 2
        r0 = HS * (c % 2)
        s = (src[b, :, r0:r0 + HS, :]
             .reshape(C, NBI, BI, NBJ, BJ)
             .transpose(0, 1, 3, 2, 4)
             .reshape(C, NBI * NBJ, BI * BJ))
        sz = np.zeros((C, NBI * NBJ, BLKC), dtype=np.float32)
        sz[:, :, 0:64] = s[:, :, 0:64]       # pixA (mi 0..3)
        sz[:, :, 128:192] = s[:, :, 64:128]  # pixB (mi 4..7)
        tp = np.zeros((C, TH, TW), dtype=np.float32)
        lo = r0 - 2 * R
        hi = r0 + HS
        clo = max(lo, 0)
        tp[:, clo - lo: clo - lo + (hi - clo), 2 * R: 2 * R + W] = \
            tgt[b, :, clo:hi, :]
        inp = np.concatenate([sz.reshape(C, SRCC),
                              tp.reshape(C, TGTC)], axis=1)
        in_maps.append({"inp": np.ascontiguousarray(inp.astype(bf16))})
    return in_maps


# host-side gather indices: out[k=(di,dj)] at pixel (mi,mj) of a block sits
# at band col n = (mi+di)*WIN_J + (mj+dj), shifted by BANDO for mi >= 4.
_mi = np.arange(BI)[:, None, None, None]
_mj = np.arange(BJ)[None, :, None, None]
_di = np.arange(D)[None, None, :, None]
_dj = np.arange(D)[None, None, None, :]
_NIDX = ((_mi + _di) * WIN_J + (_mj + _dj)
         - BANDO * (_mi >= 4)).reshape(BI, BJ, D * D)  # [8,16,81]


def _unshard_output(results):
    out = np.empty((B, D * D, H, W), dtype=np.float32)
    for c in range(NCORES):
        b = c // 2
        r0 = HS * (c % 2)
        g = (results[c]["gout"]
             .astype(np.float32)
             .reshape(NBI, NBJ // TPB, 128, TPB, BANDW)  # [bi, h, p, j, w]
             .transpose(0, 1, 3, 2, 4)
             .reshape(NBI, NBJ, BI, BJ, BANDW))
        v = np.take_along_axis(g, _NIDX[None, None], axis=-1)
        v = v.transpose(4, 0, 2, 1, 3)  # [81, NBI, BI, NBJ, BJ]
        out[b, :, r0:r0 + HS, :] = v.reshape(D * D, HS, W)
    return out


def kernel(src, tgt):
    from concourse.bass_utils import run_bass_kernel_spmd

    src = np.asarray(src, dtype=np.float32)
    tgt = np.asarray(tgt, dtype=np.float32)
    nc = _get_compiled()
    in_maps = _shard_inputs(src, tgt)
    res = run_bass_kernel_spmd(nc, in_maps, core_ids=list(range(NCORES)))
    return _unshard_output(res.results)


# revision 20
# speedup vs baseline: 1.0067x; 1.0067x over previous
"""CostVolumeLayer Trainium2 kernel (v3).

Computes the local cost volume: for search_range R=4,
  out[b, di*9+dj, i, j] = sum_c src[b,c,i,j] * tgt_zp[b,c,i-2R+di, j-2R+dj]
(tgt zero-padded outside its bounds; off-center window, faithful to the
torch reference).

Strategy (8 NeuronCores, SPMD):
  - Shard: core c -> batch b = c//2, row-half r0 = 32*(c%2).
  - Device: per 8x16 pixel block, TWO accumulating TensorE matmuls build
    the banded Gram at full 128-partition occupancy in ONE psum bank:
      A: stationary [pixels mi<4 | zeros],  rhs = window rows
         [8bi, 8bi+12) x 24 cols (N=288), start=True
      B: stationary [zeros | pixels mi>=4], rhs = rows [8bi+4, 8bi+16),
         start=False (accumulates; each writes zeros to the other half).
    The zero half comes from a single shared 64-col zero strip at the
    start of the input tile, addressed via a two-run access pattern
    (stride -pA / +pB), so src costs no duplicate bytes.
  - The DMA fabric is the bottleneck (~430 GB/s shared across ALL
    queues, measured): total traffic is 2.47 MB in + 2.36 MB out per
    core. Inputs split across the two HWDGE queues (sync: src, scalar:
    tgt); most output DMAs go through the otherwise-idle GpSimd SWDGE
    so the HWDGE queues and ACT stay free for inputs + psum copies.
  - PSUM: 4-bank tiles (4 blocks), 2 bufs = all 8 banks; one fp32->fp16
    copy per tile [128, 4x288] (DVE x5 / ACT x3); per-tile SBUF stage
    [128, 1152] -> one output DMA each (8 total).
  - PE warm-up fills the gap until the first input chunks land; first
    chunks are small so real matmuls start ~9.5us.
  - Host: zero-FLOP banded-diagonal gather (band layout identical to
    the baseline).
"""

import numpy as np

R = 4
D = 2 * R + 1          # 9
B, C, H, W = 4, 128, 64, 128
NCORES = 8
HS = H // 2            # 32 rows per core shard
TH = HS + 2 * R        # 40 padded tgt rows per shard
TW = W + 2 * R         # 136 padded tgt cols
BI, BJ = 8, 16         # pixel block: 8 rows x 16 cols = 128 = M
NBI, NBJ = HS // BI, W // BJ   # 4 x 8 = 32 blocks per core
WIN_J = BJ + 2 * R     # 24 window cols
NA = 12 * WIN_J        # 288 streamed cols per half-matmul
BANDW = NA             # 288 band cols dumped per pixel
BANDO = 4 * WIN_J      # 96, upper-half band column offset (host gather)
BLKC = 192             # src cols per block: [pixA 64 | zeros 64 | pixB 64]
SRCC = NBI * NBJ * BLKC  # 6144 src cols
TGT0 = SRCC
TGTC = TH * TW           # 5440
E = TGT0 + TGTC          # 11584 input cols per partition
PSB = 512              # fp32 elems per PSUM bank (2KB)
TPB = 4                # blocks (banks) per psum tile
STGW = TPB * BANDW     # 1152 fp16 cols per stage
NT = NBI * NBJ // TPB  # 8 psum tiles / output stages

_compiled = None


def _build_bass():
    import concourse.mybir as mybir
    from concourse import bacc
    from concourse.tile import TileContext

    f32 = mybir.dt.float32
    in_dt = mybir.dt.bfloat16
    dump_dt = mybir.dt.float16
    nc = bacc.Bacc()
    inp = nc.dram_tensor("inp", [C, E], in_dt, kind="ExternalInput")
    gout = nc.dram_tensor("gout", [NT, 128, STGW], dump_dt,
                          kind="ExternalOutput")
    gout_ap = gout.ap()

    with TileContext(nc) as tc:
        with (
            tc.tile_pool(name="inp", bufs=1) as inp_pool,
            tc.tile_pool(name="g", bufs=NT) as gpool,
            tc.tile_pool(name="psum", bufs=2, space="PSUM") as psum_pool,
        ):
            a = inp_pool.tile([C, E], in_dt)

            def t_view():
                return a[:, TGT0:].rearrange("c (i j) -> c i j", j=TW)

            def lhs_ap(blk, half):
                """Contiguous stationary windows over the per-block layout
                [pixA 64 | zeros 64 | pixB 64]: A=[pixA|z], B=[z|pixB].
                (Walrus rejects multi-run weight APs, so the zero strip is
                interleaved per block on the host instead.)"""
                sb = blk * BLKC + half * 64
                return a[:, sb:sb + 128]

            def new_pt():
                return psum_pool.tile([128, TPB * PSB], f32, name="pt")

            warm = inp_pool.tile([128, PSB], in_dt)
            nc.gpsimd.memset(warm, 0.0)
            # PE warm-up: HAM clock-gate ramp needs sustained PE activity;
            # fill the wait for the first input chunks. Warm matmuls write
            # into the first real psum tile; block 0's start=True matmul
            # overwrites (PE program order keeps this safe).
            wps = new_pt()
            for _ in range(7):
                nc.tensor.matmul(wps[0:1, 0:PSB], warm[:, :1], warm,
                                 start=True, stop=True)
            # ACT warm-up: first Activation op loads the activation table.
            actwarm = inp_pool.tile([1, 1], dump_dt)
            nc.scalar.copy(actwarm, warm[0:1, 0:1])

            # Input DMAs: ALL on the sync queue, in consumption order.
            # The DMA fabric (~430 GB/s) is shared by every queue, and one
            # queue alone saturates it — a single FIFO makes the first-
            # needed bytes complete first instead of interleaving with
            # later chunks.
            iv = inp.ap()

            def chunk(lo, hi):
                nc.sync.dma_start(out=a[:, lo:hi], in_=iv[:, lo:hi])

            def tchunk(r0, r1):
                chunk(TGT0 + r0 * TW, TGT0 + r1 * TW)

            # Chunks complete strictly in order on the queue with ~0.65us
            # per-instruction overhead: keep the critical-path chunks
            # (first tile's operands) small, later ones big.
            chunk(0, 4 * BLKC)            # blocks 0-3    (tile 0)
            tchunk(0, 12)                 # tgt rows 0-11  (bi=0 A)
            tchunk(12, 16)                # rows 12-15     (bi=0 B)
            chunk(4 * BLKC, 8 * BLKC)     # blocks 4-7    (tile 1)
            chunk(8 * BLKC, 16 * BLKC)    # blocks 8-15   (tiles 2-3)
            tchunk(16, 28)                # rows 16-27     (bi=1, bi=2 A)
            chunk(16 * BLKC, 24 * BLKC)   # blocks 16-23  (tiles 4-5)
            tchunk(28, 40)                # rows 28-39     (bi=2 B, bi=3)
            chunk(24 * BLKC, TGT0)        # blocks 24-31  (tiles 6-7)

            for t in range(NT):
                bi, h = divmod(t, 2)
                pt = wps if t == 0 else new_pt()
                ptv = pt.rearrange("p (b h) -> p b h", b=TPB)
                for j in range(TPB):
                    blk = t * TPB + j
                    bj = blk % NBJ
                    rhsA = t_view()[:, bi * BI: bi * BI + 12,
                                    bj * BJ: bj * BJ + WIN_J]
                    rhsB = t_view()[:, bi * BI + 4: bi * BI + 16,
                                    bj * BJ: bj * BJ + WIN_J]
                    nc.tensor.matmul(ptv[:, j, :NA], lhs_ap(blk, 0), rhsA,
                                     start=True, stop=False)
                    nc.tensor.matmul(ptv[:, j, :NA], lhs_ap(blk, 1), rhsB,
                                     start=False, stop=True)
                if t < NT - 1:
                    # one whole-tile copy, strict DVE/ACT alternation
                    # (splitting a tile across engines serializes: the
                    # framework orders writers of the same stage tile)
                    stage = gpool.tile([128, STGW], dump_dt)
                    dstv = stage.rearrange("p (b w) -> p b w", b=TPB)
                    eng = (nc.vector.tensor_copy if t % 2 == 0
                           else nc.scalar.copy)
                    eng(dstv, ptv[:, :, 0:BANDW])
                    # bulk outputs via the otherwise-idle GpSimd SWDGE;
                    # both HWDGE queues stay clear for inputs + the tail
                    nc.gpsimd.dma_start(out=gout_ap[t], in_=stage)
                else:
                    # tail-latency: the last tile's halves go to SEPARATE
                    # stage tiles (so DVE and ACT truly run in parallel)
                    # and ship on the two HWDGE queues concurrently
                    half = STGW // 2
                    stga = gpool.tile([128, half], dump_dt, name="stga")
                    stgb = gpool.tile([128, half], dump_dt, name="stgb")
                    nc.vector.tensor_copy(
                        stga.rearrange("p (b w) -> p b w", b=2),
                        ptv[:, 0:2, 0:BANDW])
                    nc.scalar.copy(
                        stgb.rearrange("p (b w) -> p b w", b=2),
                        ptv[:, 2:4, 0:BANDW])
                    nc.sync.dma_start(out=gout_ap[t][:, 0:half], in_=stga)
                    nc.scalar.dma_start(out=gout_ap[t][:, half:], in_=stgb)
    nc.finalize()
    return nc


def _get_compiled():
    global _compiled
    if _compiled is None:
        _compiled = _build_bass()
    return _compiled


def _shard_inputs(src, tgt):
    """Per-core input maps: [zero strip | block-reordered src | padded tgt]."""
    import ml_dtypes

    bf16 = ml_dtypes.bfloat16
    in_maps = []
    for c in range(NCORES):
        b = c // 2
        r0 = HS * (c % 2)
        s = (src[b, :, r0:r0 + HS, :]
             .reshape(C, NBI, BI, NBJ, BJ)
             .transpose(0, 1, 3, 2, 4)
             .reshape(C, NBI * NBJ, BI * BJ))
        sz = np.zeros((C, NBI * NBJ, BLKC), dtype=np.float32)
        sz[:, :, 0:64] = s[:, :, 0:64]       # pixA (mi 0..3)
        sz[:, :, 128:192] = s[:, :, 64:128]  # pixB (mi 4..7)
        tp = np.zeros((C, TH, TW), dtype=np.float32)
        lo = r0 - 2 * R
        hi = r0 + HS
        clo = max(lo, 0)
        tp[:, clo - lo: clo - lo + (hi - clo), 2 * R: 2 * R + W] = \
            tgt[b, :, clo:hi, :]
        inp = np.concatenate([sz.reshape(C, SRCC),
                              tp.reshape(C, TGTC)], axis=1)
        in_maps.append({"inp": np.ascontiguousarray(inp.astype(bf16))})
    return in_maps


# host-side gather indices: out[k=(di,dj)] at pixel (mi,mj) of a block sits
# at band col n = (mi+di)*WIN_J + (mj+dj), shifted by BANDO for mi >= 4.
_mi = np.arange(BI)[:, None, None, None]
_mj = np.arange(BJ)[None, :, None, None]
_di = np.arange(D)[None, None, :, None]
_dj = np.arange(D)[None, None, None, :]
_NIDX = ((_mi + _di) * WIN_J + (_mj + _dj)
         - BANDO * (_mi >= 4)).reshape(BI, BJ, D * D)  # [8,16,81]


def _unshard_output(results):
    out = np.empty((B, D * D, H, W), dtype=np.float32)
    for c in range(NCORES):
        b = c // 2
        r0 = HS * (c % 2)
        g = (results[c]["gout"]
             .astype(np.float32)
             .reshape(NBI, NBJ // TPB, 128, TPB, BANDW)  # [bi, h, p, j, w]
             .transpose(0, 1, 3, 2, 4)
             .reshape(NBI, NBJ, BI, BJ, BANDW))
        v = np.take_along_axis(g, _NIDX[None, None], axis=-1)
        v = v.transpose(4, 0, 2, 1, 3)  # [81, NBI, BI, NBJ, BJ]
        out[b, :, r0:r0 + HS, :] = v.reshape(D * D, HS, W)
    return out


def kernel(src, tgt):
    from concourse.bass_utils import run_bass_kernel_spmd

    src = np.asarray(src, dtype=np.float32)
    tgt = np.asarray(tgt, dtype=np.float32)
    nc = _get_compiled()
    in_maps = _shard_inputs(src, tgt)
    res = run_bass_kernel_spmd(nc, in_maps, core_ids=list(range(NCORES)))
    return _unshard_output(res.results)
